# revision 17
# baseline (speedup 1.0000x reference)
"""Deformable conv (DCNv2) Bass kernel for trn2, data-parallel over batch on 8 cores.

Per-core pipeline (one batch sample per NeuronCore):
  1. x -> SBUF as bf16 "adjacent-pair table" xe[p, 2i]=xpad[i], xe[p,2i+1]=xpad[i+1]
     (zero-padded image, 1 row top/bot, 2 cols l/r), duplicated on partitions 64-127.
  2. offset/mask 3x3 convs as 9 shifted matmuls + a "ramp" matmul folding the
     h/w base grid (1024-col chunks); ACT adds bias (+ tap const), sigmoids mask.
  3. fp32 DVE chain: floor, frac, clamps -> bilinear corner scales (mask-folded,
     bf16, (left,right)-interleaved S supertile) and flat gather indices.
  4. index wrap for ap_gather: PE transposes + ONE wide selection matmul per
     (half, b) into a (b, t8, t)-packed psum tile, DVE-copied to int16 IDXW.
  5. main loop, software-pipelined: per (window, tap-pair): scale rows are
     broadcast to 64/128 partitions by SBUF->SBUF DMA with a stride-0 dup dim
     (NOT PE matmuls); one GPSIMD ap_gather (d=2, 2 taps via partition
     groups); one DVE modulated multiply per 512-pos quarter (bf16 2x) into a
     P supertile; 4 wide corner matmuls (multi-dim strided rhs) accumulate
     out[o,j] in PSUM across the 5 tap-pairs; ACT copies psum->bf16, DMA out.

Perf history (real NTFF profiles, core 0):
  baseline (selbc matmuls + 16-col permutes): 2374 us harness / 4217 us profiled
    - PE-bound: 4193 MATMUL + 4224 LDWEIGHTS = 2.27 ms engine time, ~450 ns
      fixed overhead per matmul pair; gather only ~296 us total.
  this version: cuts PE instruction count ~4.6x (2944 -> ~640 BIR matmuls).
"""
import sys

for _p in ("/opt/trn_rl_repo", "/opt/pypackages"):
    if _p not in sys.path:
        sys.path.append(_p)

import numpy as np
import ml_dtypes

BF16 = ml_dtypes.bfloat16

B, C, H, W = 8, 64, 128, 128
OUT, K = 128, 9
NCORES = 8
NPAIR = 5  # 4 real tap pairs + (tap8, dup-tap8-with-zero-weights)


def _params(h, w):
    hw = h * w
    d = dict(H=h, W=w, HW=hw, PH=h + 2, PW=w + 4, NCH=hw // 512,
             NPASS=max(1, min(8, (hw // 512) // 4)), NG=4,
             GCH=2048 if hw >= 2048 else hw, RPC=512 // w)
    d["NE"] = d["PH"] * d["PW"]
    d["QW"] = hw // d["NG"] // d["NPASS"]
    d["CPP"] = d["NCH"] // d["NPASS"]
    d["RPK"] = 1024 // w          # image rows per 1024-col conv chunk
    d["NCHK"] = d["NCH"] // 2     # 1024-col conv chunks total
    return d


def _tap_of(pair, half):
    t = 2 * pair + half
    return 8 if t > 8 else t


def build_xe(x, h=H, w=W):
    """bf16 adjacent-pair table of the zero-padded image: [C, 2*NE]."""
    P = _params(h, w)
    PH, PW, NE = P["PH"], P["PW"], P["NE"]
    xpad = np.zeros((C, PH, PW), np.float32)
    xpad[:, 1:1 + h, 2:2 + w] = x
    flat = np.concatenate([xpad.reshape(C, NE),
                           np.zeros((C, 1), np.float32)], axis=1)
    xe = np.stack([flat[:, :NE], flat[:, 1:NE + 1]], axis=-1)  # [C, NE, 2]
    return xe.reshape(C, 2 * NE).astype(BF16)


def host_consts(w_offset, b_offset, w_mask, b_mask, w_conv, h=H, w=W):
    P = _params(h, w)
    ky = np.repeat(np.arange(3), 3).astype(np.int64)
    kx = np.tile(np.arange(3), 3).astype(np.int64)

    # conv output rows padded to quadrant bases: gy 0-8, gx 32-40, m 64-72
    WOM = np.zeros((C, 9 * 96), np.float32)
    for t in range(9):
        for k in range(9):
            WOM[:, 96 * t + k] = w_offset[2 * k, :, ky[t], kx[t]]
            WOM[:, 96 * t + 32 + k] = w_offset[2 * k + 1, :, ky[t], kx[t]]
            WOM[:, 96 * t + 64 + k] = w_mask[k, :, ky[t], kx[t]]

    RL = np.zeros((3, P["NCH"] * 96), np.float32)
    for c in range(P["NCH"]):
        RL[0, 96 * c: 96 * c + 9] = float(c * P["RPC"])  # gy += h0
        RL[1, 96 * c: 96 * c + 9] = 1.0                  # gy += hsub
        RL[2, 96 * c + 32: 96 * c + 41] = 1.0            # gx += wsub
    j = np.arange(512)
    R3 = np.stack([np.ones(512, np.float32),
                   (j // w).astype(np.float32),
                   (j % w).astype(np.float32)])

    BGY = (b_offset[0::2] + ky - 1.0).astype(np.float32).reshape(9, 1)
    BGX = (b_offset[1::2] + kx - 1.0).astype(np.float32).reshape(9, 1)
    BM = b_mask.astype(np.float32).reshape(9, 1)

    WCONV = np.zeros((128, (NPAIR + 1) * 128), np.float32)
    wc3 = w_conv.reshape(OUT, C, 9)
    for p in range(NPAIR):
        for half in range(2):
            t = 2 * p + half
            if t > 8:
                continue
            WCONV[half * 64:half * 64 + 64, 128 * p:128 * p + 128] = wc3[:, :, t].T
    WCONV[64:128, 128 * NPAIR:128 * (NPAIR + 1)] = wc3[:, :, 8].T
    IDENT = np.eye(128, dtype=np.float32)
    SEL = np.zeros((128, 8 * 128), np.float32)
    for b_ in range(8):
        for qp in range(128):
            SEL[16 * b_ + qp % 16, 128 * b_ + qp] = 1.0
    return {
        "wom": WOM.astype(BF16), "rl": RL.astype(BF16), "r3": R3.astype(BF16),
        "bgy": BGY, "bgx": BGX, "bm": BM,
        "wconv": WCONV.astype(BF16), "ident": IDENT, "sel": SEL,
    }


def emit(nc, tc, mybir, dram, h=H, w=W):
    P = _params(h, w)
    HW, PH, PW, NE = P["HW"], P["PH"], P["PW"], P["NE"]
    NPASS, QW, GCH, RPC = P["NPASS"], P["QW"], P["GCH"], P["RPC"]
    f32, bf16, i16 = mybir.dt.float32, mybir.dt.bfloat16, mybir.dt.int16
    AF = mybir.ActivationFunctionType
    OP = mybir.AluOpType
    from concourse.ap import AP as APc
    MAGIC = 12582912.0  # 1.5 * 2^23: fp32 round-to-nearest-int trick

    from contextlib import ExitStack
    ctx = ExitStack()
    sbC = ctx.enter_context(tc.tile_pool(name="sbC", bufs=1))   # persistents
    sbS = ctx.enter_context(tc.tile_pool(name="sbS", bufs=2))   # S supertiles
    sbX = ctx.enter_context(tc.tile_pool(name="sbX", bufs=1))   # chain tensors
    sbB = ctx.enter_context(tc.tile_pool(name="sbB", bufs=5))   # sb broadcast
    sbG = ctx.enter_context(tc.tile_pool(name="sbG", bufs=2))   # gather bufs
    sbP = ctx.enter_context(tc.tile_pool(name="sbP", bufs=2))   # P supertiles
    sbO = ctx.enter_context(tc.tile_pool(name="sbO", bufs=1))   # out stage
    psA = ctx.enter_context(tc.tile_pool(name="psA", bufs=2, space="PSUM"))
    psB = ctx.enter_context(tc.tile_pool(name="psB", bufs=1, space="PSUM"))

    # ---- persistent SBUF ----
    xe = sbC.tile([128, 2 * NE], bf16, tag="xe")
    IDXW = sbC.tile([128, 10 * (HW // 16)], i16, tag="IDXW")
    womt = sbC.tile([C, 9 * 96], bf16, tag="womt")
    rlt = sbC.tile([3, P["NCH"] * 96], bf16, tag="rlt")
    r3t = sbC.tile([3, 512], bf16, tag="r3t")
    bgyt = sbC.tile([9, 1], f32, tag="bgyt")
    bgxt = sbC.tile([9, 1], f32, tag="bgxt")
    bmt = sbC.tile([9, 1], f32, tag="bmt")
    wconvt = sbC.tile([128, (NPAIR + 1) * 128], bf16, tag="wconvt")
    identt = sbC.tile([128, 128], f32, tag="identt")
    selt = sbC.tile([128, 8 * 128], f32, tag="selt")

    for name, t in [("wom", womt), ("rl", rlt), ("r3", r3t), ("bgy", bgyt),
                    ("bgx", bgxt), ("bm", bmt), ("wconv", wconvt),
                    ("ident", identt), ("sel", selt)]:
        nc.sync.dma_start(out=t[:], in_=dram[name][:])

    nc.sync.dma_start(out=xe[0:64, :], in_=dram["xe"][:, :])
    nc.sync.dma_start(out=xe[64:128, :], in_=dram["xe"][:, :])
    xe3 = xe[:].rearrange("p (ph rest) -> p ph rest", ph=PH)

    # ================= per-pass: conv + chain + wrap =================
    # chain layout: quarter-group r lives at partitions [32r, 32r+9) (taps);
    # y-quantity in cols [0, QW), x-quantity in cols [QW, 2QW)
    SW = (HW // NPASS) // 16
    assert (HW // NPASS) == GCH, "gw window must equal one pass's s-range"
    assert QW == 512
    Sstore = {}

    def emit_preamble(ps):
        GYX2 = sbX.tile([128, 2 * QW], f32, tag="GYX2")
        M = sbX.tile([128, QW], f32, tag="M")
        S = sbS.tile([128, 4 * QW], bf16, tag="S")
        nc.vector.memset(GYX2[:], 0.0)
        nc.vector.memset(M[:], 0.0)
        for cw in range(4):
            cg = ps * 4 + cw
            r = cw
            hr0 = cg * RPC
            pc = psA.tile([128, 1024], f32, tag="big", name="pcbig")[0:96, 0:512]
            for t in range(9):
                tky, tkx = t // 3, t % 3
                cb = 2 * (tkx + 1)
                rhs = xe3[0:64, hr0 + tky: hr0 + tky + RPC, cb:cb + 2 * w:2]
                nc.tensor.matmul(out=pc[:, :], lhsT=womt[:, 96 * t:96 * t + 96],
                                 rhs=rhs, start=(t == 0), stop=False)
            nc.tensor.matmul(out=pc[:, :], lhsT=rlt[:, 96 * cg:96 * cg + 96],
                             rhs=r3t[:, :], start=False, stop=True)
            nc.scalar.activation(out=GYX2[32 * r:32 * r + 9, 0:512],
                                 in_=pc[0:9, :], func=AF.Identity, bias=bgyt[:, :])
            nc.scalar.activation(out=GYX2[32 * r:32 * r + 9, QW:QW + 512],
                                 in_=pc[32:41, :], func=AF.Identity, bias=bgxt[:, :])
            nc.scalar.activation(out=M[32 * r:32 * r + 9, 0:512],
                                 in_=pc[64:73, :], func=AF.Sigmoid, bias=bmt[:, :])

        # ---- chain ----
        RYX2 = sbX.tile([128, 2 * QW], f32, tag="RYX2")
        TYX2 = sbX.tile([128, 2 * QW], f32, tag="TYX2")
        WYX2 = sbX.tile([128, 2 * QW], f32, tag="WYX2")
        nc.vector.tensor_scalar(out=RYX2[:], in0=GYX2[:], scalar1=MAGIC,
                                scalar2=MAGIC, op0=OP.add, op1=OP.subtract)
        nc.vector.tensor_tensor(out=TYX2[:], in0=RYX2[:], in1=GYX2[:], op=OP.is_gt)
        nc.vector.tensor_tensor(out=TYX2[:], in0=RYX2[:], in1=TYX2[:], op=OP.subtract)
        nc.vector.tensor_tensor(out=WYX2[:], in0=GYX2[:], in1=TYX2[:], op=OP.subtract)
        OMYX2 = RYX2
        nc.vector.tensor_scalar(out=OMYX2[:], in0=WYX2[:], scalar1=-1.0,
                                scalar2=1.0, op0=OP.mult, op1=OP.add)
        A = sbX.tile([128, QW], f32, tag="A")
        Bt = sbX.tile([128, QW], f32, tag="Bt")
        nc.vector.tensor_tensor(out=A[:], in0=M[:], in1=OMYX2[:, 0:QW], op=OP.mult)
        nc.vector.tensor_tensor(out=Bt[:], in0=M[:], in1=WYX2[:, 0:QW], op=OP.mult)
        # S supertile: cols [0,2QW) = top (q,lr), [2QW,4QW) = bottom (q,lr)
        s1v = S[:, 0:2 * QW].rearrange("p (q two) -> p q two", two=2)
        s2v = S[:, 2 * QW:4 * QW].rearrange("p (q two) -> p q two", two=2)
        nc.vector.tensor_tensor(out=s1v[:, :, 0:1], in0=A[:], in1=OMYX2[:, QW:], op=OP.mult)
        nc.vector.tensor_tensor(out=s1v[:, :, 1:2], in0=A[:], in1=WYX2[:, QW:], op=OP.mult)
        nc.vector.tensor_tensor(out=s2v[:, :, 0:1], in0=Bt[:], in1=OMYX2[:, QW:], op=OP.mult)
        nc.vector.tensor_tensor(out=s2v[:, :, 1:2], in0=Bt[:], in1=WYX2[:, QW:], op=OP.mult)
        PYX0 = WYX2
        nc.vector.tensor_scalar(out=PYX0[:, 0:QW], in0=TYX2[:, 0:QW], scalar1=1.0,
                                scalar2=0.0, op0=OP.add, op1=OP.max)
        nc.vector.tensor_scalar(out=PYX0[:, 0:QW], in0=PYX0[:, 0:QW],
                                scalar1=float(h + 1), scalar2=0.0, op0=OP.min, op1=OP.add)
        nc.vector.tensor_scalar(out=PYX0[:, QW:], in0=TYX2[:, QW:], scalar1=2.0,
                                scalar2=0.0, op0=OP.add, op1=OP.max)
        nc.vector.tensor_scalar(out=PYX0[:, QW:], in0=PYX0[:, QW:],
                                scalar1=float(w + 3), scalar2=0.0, op0=OP.min, op1=OP.add)
        PY1 = A
        nc.vector.tensor_scalar(out=PY1[:], in0=TYX2[:, 0:QW], scalar1=2.0,
                                scalar2=0.0, op0=OP.add, op1=OP.max)
        nc.vector.tensor_scalar(out=PY1[:], in0=PY1[:], scalar1=float(h + 1),
                                scalar2=0.0, op0=OP.min, op1=OP.add)
        ITOP = Bt
        IBOT = M
        nc.vector.scalar_tensor_tensor(out=ITOP[:], in0=PYX0[:, 0:QW], scalar=float(PW),
                                       in1=PYX0[:, QW:], op0=OP.mult, op1=OP.add)
        nc.vector.scalar_tensor_tensor(out=IBOT[:], in0=PY1[:], scalar=float(PW),
                                       in1=PYX0[:, QW:], op0=OP.mult, op1=OP.add)

        # ---- wrap: transposes + TW2 repack + one wide permute per (half,b) ----
        # TW2[half][p, 16*t8 + 4*r + z] = I_{rc}[32r + tap(pr,half), z*128 + p]
        # with t8 = 2*pr + rc.
        TW2 = [sbX.tile([128, 160], f32, tag=f"TW2_{hf}", name=f"TW2_{hf}")
               for hf in range(2)]
        for q0 in range(0, 4, 2):
            ptp = psA.tile([128, 1024], f32, tag="big", name="ptpbig")[:, 0:512]
            for k in range(2):
                z = q0 + k
                nc.tensor.transpose(out=ptp[:, k * 256:k * 256 + 128],
                                    in_=ITOP[:, z * 128:z * 128 + 128],
                                    identity=identt[:, :])
                nc.tensor.transpose(out=ptp[:, k * 256 + 128:k * 256 + 256],
                                    in_=IBOT[:, z * 128:z * 128 + 128],
                                    identity=identt[:, :])
            ptpa = ptp[:]
            for half in range(2):
                for rc in range(2):
                    # pr<4 block: dims (pr,r,k); src col = k*256+rc*128+32r+2pr+half
                    src = APc(ptpa.tensor, ptpa.offset + (rc * 128 + half) * ptpa.ap[-1][0],
                              [ptpa.ap[0], [2, 4], [32, 4], [256, 2]])
                    dT = TW2[half][:]
                    dst = APc(dT.tensor, dT.offset + (16 * rc + q0) * dT.ap[-1][0],
                              [dT.ap[0], [32, 4], [4, 4], [1, 2]])
                    nc.vector.tensor_copy(out=dst, in_=src)
                    # pr4 (tap 8): dims (r,k); t8 = 8+rc
                    src4 = APc(ptpa.tensor, ptpa.offset + (rc * 128 + 8) * ptpa.ap[-1][0],
                               [ptpa.ap[0], [32, 4], [256, 2]])
                    dst4 = APc(dT.tensor, dT.offset + ((8 + rc) * 16 + q0) * dT.ap[-1][0],
                               [dT.ap[0], [4, 4], [1, 2]])
                    nc.vector.tensor_copy(out=dst4, in_=src4)

        # wide permutes: per (half, b) one 128-col (t8 0-7) + one 32-col (t8 8-9)
        pwA = psA.tile([128, 1024], f32, tag="big", name="pwA")
        pwB = psA.tile([128, 1024], f32, tag="big", name="pwB")
        for half in range(2):
            for b_ in range(8):
                lw = selt[:, 128 * b_ + 64 * half:128 * b_ + 64 * half + 64]
                nc.tensor.matmul(out=pwA[64 * half:64 * half + 64,
                                         b_ * 128:b_ * 128 + 128],
                                 lhsT=lw, rhs=TW2[half][:, 0:128],
                                 start=True, stop=True, skip_group_check=True)
                nc.tensor.matmul(out=pwB[64 * half:64 * half + 64,
                                         b_ * 32:b_ * 32 + 32],
                                 lhsT=lw, rhs=TW2[half][:, 128:160],
                                 start=True, stop=True, skip_group_check=True)

        # IDXW copies (f32 psum -> int16, strided)
        for pr in range(4):
            for rc in range(2):
                t8 = 2 * pr + rc
                db = 2 * pr * (HW // 16) + ps * 256 + rc * 128
                pA = pwA[:]
                src = APc(pA.tensor, pA.offset + t8 * 16 * pA.ap[-1][0],
                          [pA.ap[0], [1, 16], [128, 8]])
                dI = IDXW[:]
                dst = APc(dI.tensor, dI.offset + db * dI.ap[-1][0],
                          [dI.ap[0], [8, 16], [1, 8]])
                nc.vector.tensor_copy(out=dst, in_=src)
        for rc in range(2):
            db = 8 * (HW // 16) + ps * 256 + rc * 64
            for hf in range(2):
                pB = pwB[64 * hf:64 * hf + 64, :]
                src = APc(pB.tensor, pB.offset + (rc * 16 + 8 * hf) * pB.ap[-1][0],
                          [pB.ap[0], [1, 8], [32, 8]])
                dI = IDXW[64 * hf:64 * hf + 64, :]
                dst = APc(dI.tensor, dI.offset + db * dI.ap[-1][0],
                          [dI.ap[0], [8, 8], [1, 8]])
                nc.vector.tensor_copy(out=dst, in_=src)

        Sstore[ps] = S

    emit_preamble(0)
    for ps in range(NPASS):
        S = Sstore.pop(ps)
        Sap = S[:]
        pouts = [psB.tile([128, 512], f32, tag=f"out{ch}", name=f"pout{ch}")
                 for ch in range(4)]
        for pr in range(NPAIR):
            # ---- scale broadcast DMAs (stride-0 dup dim) ----
            sbts = []
            for cg in range(4):
                sbt = sbB.tile([128, 4 * QW], bf16, tag="sb")
                if pr < 4:
                    row = 32 * cg + 2 * pr
                    src = APc(Sap.tensor, Sap.offset + row * Sap.ap[0][0],
                              [[Sap.ap[0][0], 2], [0, 64], [1, 4 * QW]])
                    nc.sync.dma_start(out=sbt[:], in_=src)
                else:
                    row = 32 * cg + 8
                    src = APc(Sap.tensor, Sap.offset + row * Sap.ap[0][0],
                              [[Sap.ap[0][0], 1], [0, 64], [1, 4 * QW]])
                    rs0 = 64 * (cg // 2)
                    nc.sync.dma_start(out=sbt[rs0:rs0 + 64, :], in_=src)
                sbts.append(sbt)

            # ---- gather ----
            gall = sbG.tile([128, 4 * GCH], bf16, tag="gall")
            base = 2 * pr * (HW // 16) + ps * (2 * GCH // 16)
            nidx = 2 * GCH if pr < 4 else GCH
            nc.gpsimd.ap_gather(
                out_ap=gall[:, 0:2 * nidx], in_ap=xe[:],
                idxs_ap=IDXW[:, base:base + nidx // 16],
                channels=128, num_elems=NE, d=2, num_idxs=nidx)
            gal = gall[:]

            # ---- modulated multiply into P half-tiles (2 cgs each) ----
            Phs = []
            for ph in range(2):
                Pt = sbP.tile([128, 4096], bf16, tag="P")
                Phs.append(Pt)
                for cg2 in range(2):
                    cg = 2 * ph + cg2
                    if pr < 4:
                        src = APc(gal.tensor, gal.offset + cg * 1024 * gal.ap[-1][0],
                                  [gal.ap[0], [4096, 2], [1, 1024]])
                        nc.vector.tensor_tensor(
                            out=Pt[:, cg2 * 2048:cg2 * 2048 + 2048],
                            in0=src, in1=sbts[cg][:], op=OP.mult)
                    else:
                        rs0 = 64 * ph
                        g4 = gall[rs0:rs0 + 64, :]
                        src = APc(g4.tensor, g4.offset + cg2 * 1024 * g4.ap[-1][0],
                                  [g4.ap[0], [2048, 2], [1, 1024]])
                        nc.vector.tensor_tensor(
                            out=Pt[rs0:rs0 + 64, cg2 * 2048:cg2 * 2048 + 2048],
                            in0=src, in1=sbts[cg][rs0:rs0 + 64, :], op=OP.mult)

            # ---- corner matmuls accumulating out (512-col psum-bank grain) ----
            if pr < 4:
                lw = wconvt[:, 128 * pr:128 * pr + 128]
                for cg in range(4):
                    Pa = Phs[cg // 2][:]
                    for ci in range(4):
                        tb, lr = ci // 2, ci % 2
                        rhs = APc(Pa.tensor,
                                  Pa.offset + ((cg % 2) * 2048 + tb * 1024 + lr)
                                  * Pa.ap[-1][0],
                                  [Pa.ap[0], [2, 512]])
                        nc.tensor.matmul(out=pouts[cg][:], lhsT=lw, rhs=rhs,
                                         start=(pr == 0 and ci == 0),
                                         stop=False, skip_group_check=True)
            else:
                for cg in range(4):
                    if cg < 2:
                        lw = wconvt[0:64, 128 * 4:128 * 5]
                    else:
                        lw = wconvt[64:128, 128 * 5:128 * 6]
                    rs0 = 64 * (cg // 2)
                    Ph = Phs[cg // 2][rs0:rs0 + 64, :]
                    for ci in range(4):
                        tb, lr = ci // 2, ci % 2
                        rhs = APc(Ph.tensor,
                                  Ph.offset + ((cg % 2) * 2048 + tb * 1024 + lr)
                                  * Ph.ap[-1][0],
                                  [Ph.ap[0], [2, 512]])
                        nc.tensor.matmul(out=pouts[cg][:], lhsT=lw, rhs=rhs,
                                         start=False,
                                         stop=(ci == 3),
                                         skip_group_check=True)
            if pr == 0 and ps + 1 < NPASS:
                emit_preamble(ps + 1)

        oc = sbO.tile([128, 2048], bf16, tag="oc")
        for ch in range(4):
            nc.scalar.activation(out=oc[:, ch * 512:ch * 512 + 512],
                                 in_=pouts[ch][:], func=AF.Copy)
        nc.sync.dma_start(out=dram["out"][:, ps * 2048:(ps + 1) * 2048],
                          in_=oc[:])

    ctx.close()


def build_program(h=H, w=W, num_devices=NCORES):
    from concourse import bacc, mybir, tile

    nc = bacc.Bacc("TRN2", target_bir_lowering=False, debug=False,
                   num_devices=num_devices)
    P = _params(h, w)
    dram = {}

    def din(name, shape, np_dtype):
        dram[name] = nc.dram_tensor(name, list(shape), mybir.dt.from_np(np.dtype(np_dtype)),
                                    kind="ExternalInput").ap()

    din("xe", (C, 2 * P["NE"]), BF16)
    din("wom", (C, 9 * 96), BF16)
    din("rl", (3, P["NCH"] * 96), BF16)
    din("r3", (3, 512), BF16)
    din("bgy", (9, 1), np.float32)
    din("bgx", (9, 1), np.float32)
    din("bm", (9, 1), np.float32)
    din("wconv", (128, (NPAIR + 1) * 128), BF16)
    din("ident", (128, 128), np.float32)
    din("sel", (128, 8 * 128), np.float32)
    dram["out"] = nc.dram_tensor("out", [OUT, h * w], mybir.dt.bfloat16,
                                 kind="ExternalOutput").ap()
    with tile.TileContext(nc) as tc:
        emit(nc, tc, mybir, dram, h=h, w=w)
    nc.compile()
    return nc


_CACHE = {}


def kernel(x, w_offset, b_offset, w_mask, b_mask, w_conv):
    from concourse.bass_utils import run_bass_kernel_spmd

    x = np.asarray(x)
    consts = host_consts(np.asarray(w_offset), np.asarray(b_offset),
                         np.asarray(w_mask), np.asarray(b_mask),
                         np.asarray(w_conv))
    if "nc" not in _CACHE:
        _CACHE["nc"] = build_program()
    nc = _CACHE["nc"]
    in_maps = []
    for b in range(B):
        m = {"xe": build_xe(x[b].astype(np.float32))}
        m.update(consts)
        in_maps.append(m)
    res = run_bass_kernel_spmd(nc, in_maps, list(range(NCORES)))
    out = np.stack([res.results[b]["out"].astype(np.float32).reshape(OUT, H, W)
                    for b in range(B)])
    return out


# revision 31
# speedup vs baseline: 1.1709x; 1.1709x over previous
"""Deformable conv (DCNv2) Bass kernel for trn2, data-parallel over batch on 8 cores.

Per-core pipeline (one batch sample per NeuronCore):
  1. x -> SBUF as bf16 "adjacent-pair table" xe[p, 2i]=xpad[i], xe[p,2i+1]=xpad[i+1]
     (zero-padded image, 1 row top/bot, 2 cols l/r), duplicated on partitions 64-127.
  2. offset/mask 3x3 convs as 9 shifted matmuls + a "ramp" matmul folding the
     h/w base grid (1024-col chunks); ACT adds bias (+ tap const), sigmoids mask.
  3. fp32 DVE chain: floor, frac, clamps -> bilinear corner scales (mask-folded,
     bf16, (left,right)-interleaved S supertile) and flat gather indices.
  4. index wrap for ap_gather: PE transposes + ONE wide selection matmul per
     (half, b) into a (b, t8, t)-packed psum tile, DVE-copied to int16 IDXW.
  5. main loop, software-pipelined: per (window, tap-pair): scale rows are
     broadcast to 64/128 partitions by SBUF->SBUF DMA with a stride-0 dup dim
     (NOT PE matmuls); one GPSIMD ap_gather (d=2, 2 taps via partition
     groups); one DVE modulated multiply per 512-pos quarter (bf16 2x) into a
     P supertile; 4 wide corner matmuls (multi-dim strided rhs) accumulate
     out[o,j] in PSUM across the 5 tap-pairs; ACT copies psum->bf16, DMA out.

Perf history (real NTFF profiles, core 0):
  baseline (selbc matmuls + 16-col permutes): 2374 us harness / 4217 us profiled
    - PE-bound: 4193 MATMUL + 4224 LDWEIGHTS = 2.27 ms engine time, ~450 ns
      fixed overhead per matmul pair; gather only ~296 us total.
  this version: cuts PE instruction count ~4.6x (2944 -> ~640 BIR matmuls).
"""
import sys

for _p in ("/opt/trn_rl_repo", "/opt/pypackages"):
    if _p not in sys.path:
        sys.path.append(_p)

import numpy as np
import ml_dtypes

BF16 = ml_dtypes.bfloat16

B, C, H, W = 8, 64, 128, 128
OUT, K = 128, 9
NCORES = 8
NPAIR = 5  # 4 real tap pairs + (tap8, dup-tap8-with-zero-weights)


def _params(h, w):
    hw = h * w
    d = dict(H=h, W=w, HW=hw, PH=h + 2, PW=w + 4, NCH=hw // 512,
             NPASS=max(1, min(8, (hw // 512) // 4)), NG=4,
             GCH=2048 if hw >= 2048 else hw, RPC=512 // w)
    d["NE"] = d["PH"] * d["PW"]
    d["QW"] = hw // d["NG"] // d["NPASS"]
    d["CPP"] = d["NCH"] // d["NPASS"]
    d["RPK"] = 1024 // w          # image rows per 1024-col conv chunk
    d["NCHK"] = d["NCH"] // 2     # 1024-col conv chunks total
    return d


def _tap_of(pair, half):
    t = 2 * pair + half
    return 8 if t > 8 else t


def build_xe(x, h=H, w=W):
    """bf16 adjacent-pair table of the zero-padded image: [C, 2*NE]."""
    P = _params(h, w)
    PH, PW, NE = P["PH"], P["PW"], P["NE"]
    xpad = np.zeros((C, PH, PW), np.float32)
    xpad[:, 1:1 + h, 2:2 + w] = x
    flat = np.concatenate([xpad.reshape(C, NE),
                           np.zeros((C, 1), np.float32)], axis=1)
    xe = np.stack([flat[:, :NE], flat[:, 1:NE + 1]], axis=-1)  # [C, NE, 2]
    return xe.reshape(C, 2 * NE).astype(BF16)


def host_consts(w_offset, b_offset, w_mask, b_mask, w_conv, h=H, w=W):
    P = _params(h, w)
    ky = np.repeat(np.arange(3), 3).astype(np.int64)
    kx = np.tile(np.arange(3), 3).astype(np.int64)

    # conv output rows padded to quadrant bases: gy 0-8, gx 32-40, m 64-72
    WOM = np.zeros((C, 9 * 96), np.float32)
    for t in range(9):
        for k in range(9):
            WOM[:, 96 * t + k] = w_offset[2 * k, :, ky[t], kx[t]]
            WOM[:, 96 * t + 32 + k] = w_offset[2 * k + 1, :, ky[t], kx[t]]
            WOM[:, 96 * t + 64 + k] = w_mask[k, :, ky[t], kx[t]]

    RL = np.zeros((3, P["NCH"] * 96), np.float32)
    for c in range(P["NCH"]):
        RL[0, 96 * c: 96 * c + 9] = float(c * P["RPC"])  # gy += h0
        RL[1, 96 * c: 96 * c + 9] = 1.0                  # gy += hsub
        RL[2, 96 * c + 32: 96 * c + 41] = 1.0            # gx += wsub
    j = np.arange(512)
    R3 = np.stack([np.ones(512, np.float32),
                   (j // w).astype(np.float32),
                   (j % w).astype(np.float32)])

    BGY = (b_offset[0::2] + ky - 1.0).astype(np.float32).reshape(9, 1)
    BGX = (b_offset[1::2] + kx - 1.0).astype(np.float32).reshape(9, 1)
    BM = b_mask.astype(np.float32).reshape(9, 1)

    WCONV = np.zeros((128, (NPAIR + 1) * 128), np.float32)
    wc3 = w_conv.reshape(OUT, C, 9)
    for p in range(NPAIR):
        for half in range(2):
            t = 2 * p + half
            if t > 8:
                continue
            WCONV[half * 64:half * 64 + 64, 128 * p:128 * p + 128] = wc3[:, :, t].T
    WCONV[64:128, 128 * NPAIR:128 * (NPAIR + 1)] = wc3[:, :, 8].T
    IDENT = np.eye(128, dtype=np.float32)
    SEL = np.zeros((128, 8 * 128), np.float32)
    for b_ in range(8):
        for qp in range(128):
            SEL[16 * b_ + qp % 16, 128 * b_ + qp] = 1.0
    return {
        "wom": WOM.astype(BF16), "rl": RL.astype(BF16), "r3": R3.astype(BF16),
        "bgy": BGY, "bgx": BGX, "bm": BM,
        "wconv": WCONV.astype(BF16), "ident": IDENT, "sel": SEL,
    }


def emit(nc, tc, mybir, dram, h=H, w=W):
    P = _params(h, w)
    HW, PH, PW, NE = P["HW"], P["PH"], P["PW"], P["NE"]
    NPASS, QW, GCH, RPC = P["NPASS"], P["QW"], P["GCH"], P["RPC"]
    f32, bf16, i16 = mybir.dt.float32, mybir.dt.bfloat16, mybir.dt.int16
    AF = mybir.ActivationFunctionType
    OP = mybir.AluOpType
    from concourse.ap import AP as APc
    MAGIC = 12582912.0  # 1.5 * 2^23: fp32 round-to-nearest-int trick

    from contextlib import ExitStack
    ctx = ExitStack()
    sbC = ctx.enter_context(tc.tile_pool(name="sbC", bufs=1))   # persistents
    sbS = ctx.enter_context(tc.tile_pool(name="sbS", bufs=2))   # S supertiles
    sbX = ctx.enter_context(tc.tile_pool(name="sbX", bufs=1))   # chain tensors
    sbB = ctx.enter_context(tc.tile_pool(name="sbB", bufs=5))   # sb broadcast
    sbSP = ctx.enter_context(tc.tile_pool(name="sbSP", bufs=5))  # 16x-dup stage
    sbG = ctx.enter_context(tc.tile_pool(name="sbG", bufs=2))   # gather bufs
    sbP = ctx.enter_context(tc.tile_pool(name="sbP", bufs=2))   # P supertiles
    sbO = ctx.enter_context(tc.tile_pool(name="sbO", bufs=1))   # out stage
    psA = ctx.enter_context(tc.tile_pool(name="psA", bufs=2, space="PSUM"))
    psB = ctx.enter_context(tc.tile_pool(name="psB", bufs=1, space="PSUM"))

    # ---- persistent SBUF ----
    xe = sbC.tile([128, 2 * NE], bf16, tag="xe")
    IDXW = sbC.tile([128, 9 * (HW // 16)], i16, tag="IDXW")
    womt = sbC.tile([C, 9 * 96], bf16, tag="womt")
    rlt = sbC.tile([3, P["NCH"] * 96], bf16, tag="rlt")
    r3t = sbC.tile([3, 512], bf16, tag="r3t")
    bgyt = sbC.tile([9, 1], f32, tag="bgyt")
    bgxt = sbC.tile([9, 1], f32, tag="bgxt")
    bmt = sbC.tile([9, 1], f32, tag="bmt")
    wconvt = sbC.tile([128, (NPAIR + 1) * 128], bf16, tag="wconvt")
    identt = sbC.tile([128, 128], f32, tag="identt")
    selt = sbC.tile([128, 8 * 128], f32, tag="selt")

    for name, t in [("wom", womt), ("rl", rlt), ("r3", r3t), ("bgy", bgyt),
                    ("bgx", bgxt), ("bm", bmt), ("wconv", wconvt),
                    ("ident", identt), ("sel", selt)]:
        nc.sync.dma_start(out=t[:], in_=dram[name][:])

    nc.sync.dma_start(out=xe[0:64, :], in_=dram["xe"][:, :])
    nc.sync.dma_start(out=xe[64:128, :], in_=dram["xe"][:, :])
    xe3 = xe[:].rearrange("p (ph rest) -> p ph rest", ph=PH)

    # ================= per-pass: conv + chain + wrap =================
    # chain layout: quarter-group r lives at partitions [32r, 32r+9) (taps);
    # y-quantity in cols [0, QW), x-quantity in cols [QW, 2QW)
    SW = (HW // NPASS) // 16
    assert (HW // NPASS) == GCH, "gw window must equal one pass's s-range"
    assert QW == 512
    Sstore = {}

    def emit_preamble(ps):
        GYX2 = sbX.tile([128, 2 * QW], f32, tag="GYX2")
        M = sbX.tile([128, QW], f32, tag="M")
        S = sbS.tile([128, 4 * QW], bf16, tag="S")
        nc.vector.memset(GYX2[:], 0.0)
        nc.vector.memset(M[:], 0.0)
        # conv: tap-outer over all 4 cgs (2 cgs share one [128,1024] psum tile)
        # so consecutive matmuls share lhsT and LDWEIGHTS dedups.
        pc2 = [psA.tile([128, 1024], f32, tag="big", name=f"pc2_{i}")
               for i in range(2)]
        for t in range(9):
            tky, tkx = t // 3, t % 3
            cb = 2 * (tkx + 1)
            for cw in range(4):
                cg = ps * 4 + cw
                hr0 = cg * RPC
                rhs = xe3[0:64, hr0 + tky: hr0 + tky + RPC, cb:cb + 2 * w:2]
                pc = pc2[cw // 2][0:96, 512 * (cw % 2):512 * (cw % 2) + 512]
                nc.tensor.matmul(out=pc, lhsT=womt[:, 96 * t:96 * t + 96],
                                 rhs=rhs, start=(t == 0), stop=False,
                                 skip_group_check=True)
        for cw in range(4):
            cg = ps * 4 + cw
            pc = pc2[cw // 2][0:96, 512 * (cw % 2):512 * (cw % 2) + 512]
            nc.tensor.matmul(out=pc, lhsT=rlt[:, 96 * cg:96 * cg + 96],
                             rhs=r3t[:, :], start=False, stop=True,
                             skip_group_check=True)
        for cw in range(4):
            r = cw
            pc = pc2[cw // 2][:, 512 * (cw % 2):512 * (cw % 2) + 512]
            nc.scalar.activation(out=GYX2[32 * r:32 * r + 9, 0:512],
                                 in_=pc[0:9, :], func=AF.Identity, bias=bgyt[:, :])
            nc.scalar.activation(out=GYX2[32 * r:32 * r + 9, QW:QW + 512],
                                 in_=pc[32:41, :], func=AF.Identity, bias=bgxt[:, :])
            nc.scalar.activation(out=M[32 * r:32 * r + 9, 0:512],
                                 in_=pc[64:73, :], func=AF.Sigmoid, bias=bmt[:, :])

        # ---- chain ----
        RYX2 = sbX.tile([128, 2 * QW], f32, tag="RYX2")
        TYX2 = sbX.tile([128, 2 * QW], f32, tag="TYX2")
        WYX2 = sbX.tile([128, 2 * QW], f32, tag="WYX2")
        nc.vector.tensor_scalar(out=RYX2[:], in0=GYX2[:], scalar1=MAGIC,
                                scalar2=MAGIC, op0=OP.add, op1=OP.subtract)
        nc.vector.tensor_tensor(out=TYX2[:], in0=RYX2[:], in1=GYX2[:], op=OP.is_gt)
        nc.vector.tensor_tensor(out=TYX2[:], in0=RYX2[:], in1=TYX2[:], op=OP.subtract)
        nc.vector.tensor_tensor(out=WYX2[:], in0=GYX2[:], in1=TYX2[:], op=OP.subtract)
        OMYX2 = RYX2
        nc.vector.tensor_scalar(out=OMYX2[:], in0=WYX2[:], scalar1=-1.0,
                                scalar2=1.0, op0=OP.mult, op1=OP.add)
        A = sbX.tile([128, QW], f32, tag="A")
        Bt = sbX.tile([128, QW], f32, tag="Bt")
        nc.vector.tensor_tensor(out=A[:], in0=M[:], in1=OMYX2[:, 0:QW], op=OP.mult)
        nc.vector.tensor_tensor(out=Bt[:], in0=M[:], in1=WYX2[:, 0:QW], op=OP.mult)
        # S supertile: cols [0,2QW) = top (q,lr), [2QW,4QW) = bottom (q,lr)
        s1v = S[:, 0:2 * QW].rearrange("p (q two) -> p q two", two=2)
        s2v = S[:, 2 * QW:4 * QW].rearrange("p (q two) -> p q two", two=2)
        nc.vector.tensor_tensor(out=s1v[:, :, 0:1], in0=A[:], in1=OMYX2[:, QW:], op=OP.mult)
        nc.vector.tensor_tensor(out=s1v[:, :, 1:2], in0=A[:], in1=WYX2[:, QW:], op=OP.mult)
        nc.vector.tensor_tensor(out=s2v[:, :, 0:1], in0=Bt[:], in1=OMYX2[:, QW:], op=OP.mult)
        nc.vector.tensor_tensor(out=s2v[:, :, 1:2], in0=Bt[:], in1=WYX2[:, QW:], op=OP.mult)
        PYX0 = WYX2
        nc.vector.tensor_scalar(out=PYX0[:, 0:QW], in0=TYX2[:, 0:QW], scalar1=1.0,
                                scalar2=0.0, op0=OP.add, op1=OP.max)
        nc.vector.tensor_scalar(out=PYX0[:, 0:QW], in0=PYX0[:, 0:QW],
                                scalar1=float(h + 1), scalar2=0.0, op0=OP.min, op1=OP.add)
        nc.vector.tensor_scalar(out=PYX0[:, QW:], in0=TYX2[:, QW:], scalar1=2.0,
                                scalar2=0.0, op0=OP.add, op1=OP.max)
        nc.vector.tensor_scalar(out=PYX0[:, QW:], in0=PYX0[:, QW:],
                                scalar1=float(w + 3), scalar2=0.0, op0=OP.min, op1=OP.add)
        PY1 = A
        nc.vector.tensor_scalar(out=PY1[:], in0=TYX2[:, 0:QW], scalar1=2.0,
                                scalar2=0.0, op0=OP.add, op1=OP.max)
        nc.vector.tensor_scalar(out=PY1[:], in0=PY1[:], scalar1=float(h + 1),
                                scalar2=0.0, op0=OP.min, op1=OP.add)
        ITOP = Bt
        IBOT = M
        nc.vector.scalar_tensor_tensor(out=ITOP[:], in0=PYX0[:, 0:QW], scalar=float(PW),
                                       in1=PYX0[:, QW:], op0=OP.mult, op1=OP.add)
        nc.vector.scalar_tensor_tensor(out=IBOT[:], in0=PY1[:], scalar=float(PW),
                                       in1=PYX0[:, QW:], op0=OP.mult, op1=OP.add)

        # ---- wrap: transposes + TW2 repack + one wide permute per (half,b) ----
        # TW2[half][p, 16*t8 + 4*r + z] = I_{rc}[32r + tap(pr,half), z*128 + p]
        # with t8 = 2*pr + rc.
        TW2 = [sbX.tile([128, 160], f32, tag=f"TW2_{hf}", name=f"TW2_{hf}")
               for hf in range(2)]
        for q0 in range(0, 4, 2):
            ptp = psA.tile([128, 1024], f32, tag="big", name="ptpbig")[:, 0:512]
            for k in range(2):
                z = q0 + k
                nc.tensor.transpose(out=ptp[:, k * 256:k * 256 + 128],
                                    in_=ITOP[:, z * 128:z * 128 + 128],
                                    identity=identt[:, :])
                nc.tensor.transpose(out=ptp[:, k * 256 + 128:k * 256 + 256],
                                    in_=IBOT[:, z * 128:z * 128 + 128],
                                    identity=identt[:, :])
            ptpa = ptp[:]
            for half in range(2):
                for rc in range(2):
                    # pr<4 block: dims (pr,r,k); src col = k*256+rc*128+32r+2pr+half
                    src = APc(ptpa.tensor, ptpa.offset + (rc * 128 + half) * ptpa.ap[-1][0],
                              [ptpa.ap[0], [2, 4], [32, 4], [256, 2]])
                    dT = TW2[half][:]
                    dst = APc(dT.tensor, dT.offset + (16 * rc + q0) * dT.ap[-1][0],
                              [dT.ap[0], [32, 4], [4, 4], [1, 2]])
                    nc.vector.tensor_copy(out=dst, in_=src)
                    # pr4 (tap 8): dims (r,k); t8 = 8+rc
                    src4 = APc(ptpa.tensor, ptpa.offset + (rc * 128 + 8) * ptpa.ap[-1][0],
                               [ptpa.ap[0], [32, 4], [256, 2]])
                    dst4 = APc(dT.tensor, dT.offset + ((8 + rc) * 16 + q0) * dT.ap[-1][0],
                               [dT.ap[0], [4, 4], [1, 2]])
                    nc.vector.tensor_copy(out=dst4, in_=src4)

        # wide permutes: per (half, b) one 128-col (t8 0-7) + one 32-col (t8 8-9)
        pwA = psA.tile([128, 1024], f32, tag="big", name="pwA")
        pwB = psA.tile([128, 1024], f32, tag="big", name="pwB")
        for half in range(2):
            for b_ in range(8):
                lw = selt[:, 128 * b_ + 64 * half:128 * b_ + 64 * half + 64]
                nc.tensor.matmul(out=pwA[64 * half:64 * half + 64,
                                         b_ * 128:b_ * 128 + 128],
                                 lhsT=lw, rhs=TW2[half][:, 0:128],
                                 start=True, stop=True, skip_group_check=True)
                nc.tensor.matmul(out=pwB[64 * half:64 * half + 64,
                                         b_ * 32:b_ * 32 + 32],
                                 lhsT=lw, rhs=TW2[half][:, 128:160],
                                 start=True, stop=True, skip_group_check=True)

        # IDXW copies (f32 psum -> int16, strided).  Layout per pr block and
        # pass: [A-rc0(64) | A-rc1(64) | B-rc0(64) | B-rc1(64)] where A/B are
        # position halves (t 0-7 / 8-15), so the gather can run as two
        # half-window calls with contiguous idx slices.
        for pr in range(4):
            for rc in range(2):
                t8 = 2 * pr + rc
                for th in range(2):
                    db = (2 * pr * (HW // 16) + ps * 256 + th * 128 + rc * 64)
                    pA = pwA[:]
                    src = APc(pA.tensor,
                              pA.offset + (t8 * 16 + th * 8) * pA.ap[-1][0],
                              [pA.ap[0], [1, 8], [128, 8]])
                    dI = IDXW[:]
                    dst = APc(dI.tensor, dI.offset + db * dI.ap[-1][0],
                              [dI.ap[0], [8, 8], [1, 8]])
                    nc.vector.tensor_copy(out=dst, in_=src)
        for rc in range(2):
            db = 8 * (HW // 16) + ps * 128 + rc * 64
            for hf in range(2):
                pB = pwB[64 * hf:64 * hf + 64, :]
                src = APc(pB.tensor, pB.offset + (rc * 16 + 8 * hf) * pB.ap[-1][0],
                          [pB.ap[0], [1, 8], [32, 8]])
                dI = IDXW[64 * hf:64 * hf + 64, :]
                dst = APc(dI.tensor, dI.offset + db * dI.ap[-1][0],
                          [dI.ap[0], [8, 8], [1, 8]])
                nc.vector.tensor_copy(out=dst, in_=src)

        Sstore[ps] = S

    def emit_stage1(Sap):
        """Per-cg 16x partition-dup of all 9 tap scale rows.

        sps[cg][16*(2pr+hf) + k, :] = S[32cg + 2pr + hf, :]   (taps 0-7)
        sp4[32cg + k, :]            = S[32cg + 8, :]          (tap 8)
        All APs keep dim0 stride == row pitch so dep tracking stays exact.
        """
        pstr = Sap.ap[0][0]
        sps = []
        for cg in range(4):
            sp = sbSP.tile([128, 4 * QW], bf16, tag="sp", name=f"sp{cg}")
            src = APc(Sap.tensor, Sap.offset + 32 * cg * pstr,
                      [[pstr, 8], [0, 16], [1, 4 * QW]])
            eng = nc.sync if cg % 2 == 0 else nc.scalar
            eng.dma_start(out=sp[:], in_=src)
            sps.append(sp)
        sp4 = sbSP.tile([128, 4 * QW], bf16, tag="sp", name="sp4")
        for cg in range(4):
            src = APc(Sap.tensor, Sap.offset + (32 * cg + 8) * pstr,
                      [[pstr, 1], [0, 32], [1, 4 * QW]])
            eng = nc.sync if cg % 2 == 0 else nc.scalar
            eng.dma_start(out=sp4[32 * cg:32 * cg + 32, :], in_=src)
        return sps, sp4

    def emit_stage2(sps, sp4, pr):
        """64x fan-out reading 16 (32 for tap8) source partitions per DMA."""
        sbts = []
        for cg in range(4):
            sbt = sbB.tile([128, 4 * QW], bf16, tag="sb")
            eng = nc.sync if cg % 2 == 0 else nc.scalar
            if pr < 4:
                spa = sps[cg][:]
                pstr = spa.ap[0][0]
                # dst p = 64hf + 4k + dup  <-  sp[16*(2pr+hf) + k, :]
                for hf in range(2):
                    src = APc(spa.tensor,
                              spa.offset + 16 * (2 * pr + hf) * pstr,
                              [[pstr, 16], [0, 4], [1, 4 * QW]])
                    eng.dma_start(out=sbt[64 * hf:64 * hf + 64, :], in_=src)
            else:
                spa = sp4[:]
                pstr = spa.ap[0][0]
                src = APc(spa.tensor, spa.offset + 32 * cg * pstr,
                          [[pstr, 32], [0, 2], [1, 4 * QW]])
                rs0 = 64 * (cg // 2)
                eng.dma_start(out=sbt[rs0:rs0 + 64, :], in_=src)
            sbts.append(sbt)
        return sbts

    emit_preamble(0)
    for ps in range(NPASS):
        S = Sstore.pop(ps)
        Sap = S[:]
        pouts = [psB.tile([128, 512], f32, tag=f"out{ch}", name=f"pout{ch}")
                 for ch in range(4)]
        sps, sp4 = emit_stage1(Sap)
        sbts_cur = emit_stage2(sps, sp4, 0)
        for pr in range(NPAIR):
            sbts = sbts_cur
            # prefetch next pair's broadcasts so they overlap this pair's math
            if pr + 1 < NPAIR:
                sbts_cur = emit_stage2(sps, sp4, pr + 1)

            # ---- gather (two half-window calls for pr<4) ----
            base = (2 * pr * (HW // 16) + ps * (2 * GCH // 16)
                    if pr < 4 else 8 * (HW // 16) + ps * (GCH // 16))
            galls = []
            if pr < 4:
                for th in range(2):
                    gall = sbG.tile([128, 2 * GCH], bf16, tag="gall",
                                    name=f"gall{th}")
                    nc.gpsimd.ap_gather(
                        out_ap=gall[:, 0:2 * GCH], in_ap=xe[:],
                        idxs_ap=IDXW[:, base + th * 128:base + th * 128 + 128],
                        channels=128, num_elems=NE, d=2, num_idxs=GCH)
                    galls.append(gall)
            else:
                gall = sbG.tile([128, 2 * GCH], bf16, tag="gall", name="gall4")
                nc.gpsimd.ap_gather(
                    out_ap=gall[:, 0:2 * GCH], in_ap=xe[:],
                    idxs_ap=IDXW[:, base:base + GCH // 16],
                    channels=128, num_elems=NE, d=2, num_idxs=GCH)
                galls = [gall, gall]

            # ---- modulated multiply into P half-tiles (2 cgs each) ----
            Phs = []
            for ph in range(2):
                Pt = sbP.tile([128, 4096], bf16, tag="P")
                Phs.append(Pt)
                gal = galls[ph][:]
                for cg2 in range(2):
                    cg = 2 * ph + cg2
                    if pr < 4:
                        src = APc(gal.tensor, gal.offset + cg2 * 1024 * gal.ap[-1][0],
                                  [gal.ap[0], [2048, 2], [1, 1024]])
                        nc.vector.tensor_tensor(
                            out=Pt[:, cg2 * 2048:cg2 * 2048 + 2048],
                            in0=src, in1=sbts[cg][:], op=OP.mult)
                    else:
                        rs0 = 64 * ph
                        g4 = galls[0][rs0:rs0 + 64, :]
                        src = APc(g4.tensor, g4.offset + cg2 * 1024 * g4.ap[-1][0],
                                  [g4.ap[0], [2048, 2], [1, 1024]])
                        nc.vector.tensor_tensor(
                            out=Pt[rs0:rs0 + 64, cg2 * 2048:cg2 * 2048 + 2048],
                            in0=src, in1=sbts[cg][rs0:rs0 + 64, :], op=OP.mult)

            # ---- corner matmuls accumulating out (512-col psum-bank grain) ----
            if pr < 4:
                lw = wconvt[:, 128 * pr:128 * pr + 128]
                for cg in range(4):
                    Pa = Phs[cg // 2][:]
                    for ci in range(4):
                        tb, lr = ci // 2, ci % 2
                        rhs = APc(Pa.tensor,
                                  Pa.offset + ((cg % 2) * 2048 + tb * 1024 + lr)
                                  * Pa.ap[-1][0],
                                  [Pa.ap[0], [2, 512]])
                        nc.tensor.matmul(out=pouts[cg][:], lhsT=lw, rhs=rhs,
                                         start=(pr == 0 and ci == 0),
                                         stop=False, skip_group_check=True)
            else:
                for cg in range(4):
                    if cg < 2:
                        lw = wconvt[0:64, 128 * 4:128 * 5]
                    else:
                        lw = wconvt[64:128, 128 * 5:128 * 6]
                    rs0 = 64 * (cg // 2)
                    Ph = Phs[cg // 2][rs0:rs0 + 64, :]
                    for ci in range(4):
                        tb, lr = ci // 2, ci % 2
                        rhs = APc(Ph.tensor,
                                  Ph.offset + ((cg % 2) * 2048 + tb * 1024 + lr)
                                  * Ph.ap[-1][0],
                                  [Ph.ap[0], [2, 512]])
                        nc.tensor.matmul(out=pouts[cg][:], lhsT=lw, rhs=rhs,
                                         start=False,
                                         stop=(ci == 3),
                                         skip_group_check=True)
            if pr == 0 and ps + 1 < NPASS:
                emit_preamble(ps + 1)

        oc = sbO.tile([128, 2048], bf16, tag="oc")
        for ch in range(4):
            nc.scalar.activation(out=oc[:, ch * 512:ch * 512 + 512],
                                 in_=pouts[ch][:], func=AF.Copy)
        nc.sync.dma_start(out=dram["out"][:, ps * 2048:(ps + 1) * 2048],
                          in_=oc[:])

    ctx.close()


def build_program(h=H, w=W, num_devices=NCORES):
    from concourse import bacc, mybir, tile

    nc = bacc.Bacc("TRN2", target_bir_lowering=False, debug=False,
                   num_devices=num_devices)
    P = _params(h, w)
    dram = {}

    def din(name, shape, np_dtype):
        dram[name] = nc.dram_tensor(name, list(shape), mybir.dt.from_np(np.dtype(np_dtype)),
                                    kind="ExternalInput").ap()

    din("xe", (C, 2 * P["NE"]), BF16)
    din("wom", (C, 9 * 96), BF16)
    din("rl", (3, P["NCH"] * 96), BF16)
    din("r3", (3, 512), BF16)
    din("bgy", (9, 1), np.float32)
    din("bgx", (9, 1), np.float32)
    din("bm", (9, 1), np.float32)
    din("wconv", (128, (NPAIR + 1) * 128), BF16)
    din("ident", (128, 128), np.float32)
    din("sel", (128, 8 * 128), np.float32)
    dram["out"] = nc.dram_tensor("out", [OUT, h * w], mybir.dt.bfloat16,
                                 kind="ExternalOutput").ap()
    with tile.TileContext(nc) as tc:
        emit(nc, tc, mybir, dram, h=h, w=w)
    nc.compile()
    return nc


_CACHE = {}


def kernel(x, w_offset, b_offset, w_mask, b_mask, w_conv):
    from concourse.bass_utils import run_bass_kernel_spmd

    x = np.asarray(x)
    consts = host_consts(np.asarray(w_offset), np.asarray(b_offset),
                         np.asarray(w_mask), np.asarray(b_mask),
                         np.asarray(w_conv))
    if "nc" not in _CACHE:
        _CACHE["nc"] = build_program()
    nc = _CACHE["nc"]
    in_maps = []
    for b in range(B):
        m = {"xe": build_xe(x[b].astype(np.float32))}
        m.update(consts)
        in_maps.append(m)
    res = run_bass_kernel_spmd(nc, in_maps, list(range(NCORES)))
    out = np.stack([res.results[b]["out"].astype(np.float32).reshape(OUT, H, W)
                    for b in range(B)])
    return out
